# revision 35
# baseline (speedup 1.0000x reference)
"""Trainium2 Bass kernel for LeViT-style cross attention (nn_Attention).

Strategy: pure data-parallel over batch B=32 across 8 NeuronCores (4 per
core, no collectives).  Host precomputes the tiny shared pieces (BN folds,
the 400x2560 kv projection, exp() of the gathered relative-position bias
table) and pre-transposes layouts; each core runs the heavy ~19 GFLOP of
per-batch attention: Q projection, scores+softmax, attn@V, GELU, output
projection.

Device kernel per core, per (batch, n-half of 512), all layouts transposed
(feature dim on partitions) so no on-chip transposes are needed:
  A: Q^T = Wq^T @ X^T             (bf16 matmul, bias epilogue on DVE)
  B per head pair (heads packed at PE row groups 0/64 so the K=64 score
     matmuls row-tile the PE array and run concurrently):
       scoresT[t,n] accumulate into a 4-bank PSUM tile
       ee = exp(scoresT) (ACT, reads PSUM directly, bf16 out)
       ee *= exp(bias)   (DVE bf16 2x tensor_tensor, in-place; softmax
                          identity exp(s+b) = exp(s)*exp(b) keeps the bias
                          add off the PSUM-source fp32 1x DVE path)
     per head:
       denominator self-broadcasting: lhsT = ones[100,128] makes the PE
       write the row-sum to ALL 128 psum partitions, so the reciprocal
       (custom-DVE reciprocal_approx_fast) comes out already broadcast
       (partition_broadcast from a non-zero base partition reads the wrong
       row on HW -- probe-verified)
       out^T = V^T @ ee (bf16) -> PSUM; avn = psum * recip (DVE)
  C: gelu on one contiguous avn tile (big ACT ops, split around the last
     heads' AV so the PE bridges the gelu table-switch block), then
     proj: final^T = Wp^T @ gelu^T + folded-BN bias epilogue, DMA out
     bf16 as final^T; host transposes back / casts.

All resident weights are host-packed contiguous-per-partition so each loads
with ONE DMA (each DMA instruction costs ~800ns of serial sync-queue issue
time; 75 small DMAs was a ~50us startup stall).  Q-projection runs two
iterations ahead so iteration 0 (which has no proj work) still has dense PE
slots.
"""

import numpy as np
import ml_dtypes

# Model hyperparameters (hardcoded per spec nn_Attention_81449759801699)
B, N_TOK, DIM = 32, 1024, 512
NT = 400
NUM_HEADS, KEY_DIM = 8, 64
D_V = 256
DH = D_V * NUM_HEADS          # 2048
NH_KD = KEY_DIM * NUM_HEADS   # 512
H_KV = DH + NH_KD             # 2560
H_GRID, W_GRID = 32, 32
EPS = 1e-5
N_CORES = 8
B_LOC = B // N_CORES          # 4 batches per core
NH2 = 512                     # n-half
TC = 100                      # t-chunk (400 = 4*100)

_CACHE = {}


def _build_nc():
    """Build + compile the single-core Bass graph (same graph on all 8 cores)."""
    from contextlib import ExitStack
    import concourse.bass as bass
    import concourse.bacc as bacc
    import concourse.tile as tile
    from concourse import mybir

    f32 = mybir.dt.float32
    bf16 = mybir.dt.bfloat16
    AF = mybir.ActivationFunctionType
    ALU = mybir.AluOpType

    nc = bacc.Bacc("TRN2", target_bir_lowering=False, debug=False,
                   num_devices=N_CORES)

    # All tensors host-packed [128-partition dim, contiguous free] so every
    # load is a single DMA.
    xT_d = nc.dram_tensor("xT", [B_LOC, 2, 128, 4, NH2], bf16,
                          kind="ExternalInput")
    wq_d = nc.dram_tensor("wq", [128, 4, NH_KD], bf16, kind="ExternalInput")
    bq_d = nc.dram_tensor("bq", [128, 4], f32, kind="ExternalInput")
    kT_d = nc.dram_tensor("kT", [128, NUM_HEADS // 2, NT], bf16,
                          kind="ExternalInput")
    v_d = nc.dram_tensor("v", [TC, NUM_HEADS, 4, D_V], bf16,
                         kind="ExternalInput")
    # exp(bias): [half, t_local(100), h, chunk(4), n(512)]
    eb_d = nc.dram_tensor("expbias", [2, TC, NUM_HEADS, 4, NH2], bf16,
                          kind="ExternalInput")
    wp_d = nc.dram_tensor("wp", [128, 16, DIM], bf16, kind="ExternalInput")
    bp_d = nc.dram_tensor("bp", [128, 4], f32, kind="ExternalInput")
    out_d = nc.dram_tensor("outT", [B_LOC, 2, 128, 4, NH2], bf16,
                           kind="ExternalOutput")

    with tile.TileContext(nc) as tc, ExitStack() as ctx:
        resid = ctx.enter_context(tc.tile_pool(name="resid", bufs=1))
        xt_pool = ctx.enter_context(tc.tile_pool(name="xt", bufs=4))
        qt_pool = ctx.enter_context(tc.tile_pool(name="qt", bufs=12))
        ee_pool = ctx.enter_context(tc.tile_pool(name="ep", bufs=4))
        rbc_pool = ctx.enter_context(tc.tile_pool(name="rbcp", bufs=2))
        avn_pool = ctx.enter_context(tc.tile_pool(name="avnp", bufs=1))
        outg_pool = ctx.enter_context(tc.tile_pool(name="outgp", bufs=2))
        fin_pool = ctx.enter_context(tc.tile_pool(name="finp", bufs=2))
        ps_sc = ctx.enter_context(tc.tile_pool(name="pssc", bufs=1, space="PSUM"))
        ps_av = ctx.enter_context(tc.tile_pool(name="psav", bufs=1, space="PSUM"))
        # Qproj, proj and denominator rounds share one 2-slot bank pool so
        # consecutive rounds pipeline instead of serializing on the evac.
        ps_mm = ctx.enter_context(tc.tile_pool(name="psmm", bufs=2, space="PSUM"))

        # ---- resident weights (one DMA each, ordered by first use) ----
        wq = resid.tile([128, 4, NH_KD], bf16, name="wq", tag="wq")
        nc.sync.dma_start(out=wq[:], in_=wq_d[:, :, :])
        bq = resid.tile([128, 4], f32, name="bq", tag="bq")
        nc.sync.dma_start(out=bq[:], in_=bq_d[:, :])
        kT = resid.tile([128, NUM_HEADS // 2, NT], bf16, name="kT", tag="kT")
        nc.sync.dma_start(out=kT[:], in_=kT_d[:, :, :])

        # First three iterations' x tiles, ahead of the bulky residents.
        xts = {}
        for ei in range(3):
            t = xt_pool.tile([128, 4, NH2], bf16, name=f"xte{ei}", tag="xt")
            nc.sync.dma_start(out=t[:], in_=xT_d[ei // 2, ei % 2])
            xts[ei] = t

        # eb/v interleaved in first-use order (heads 0-3 of half 0 are
        # needed within ~15us of kernel start); x for iters 3-4 and the
        # hf=1 exp-bias table land before wp (first used an iteration
        # later) -- the qproj-two-ahead schedule consumes x(it+2) early.
        # The bulky residents are spread across four engines' DMA queues
        # (independent hardware rings, ~parallel transfer) -- one ring alone
        # delivers only ~150GB/s, which made the 12MB resident load an
        # ~80us tail that stalled iteration 1 on wp/eb1.
        eb0 = resid.tile([TC, NUM_HEADS, 4, NH2], bf16, name="eb0", tag="eb0")
        vv = resid.tile([TC, NUM_HEADS, 4, D_V], bf16, name="vv", tag="vv")
        eb1 = resid.tile([TC, NUM_HEADS, 4, NH2], bf16, name="eb1", tag="eb1")
        wp = resid.tile([128, 16, DIM], bf16, name="wp", tag="wp")
        nc.scalar.dma_start(out=eb0[0:TC, 0:4, :, :], in_=eb_d[0, :, 0:4])
        nc.gpsimd.dma_start(out=vv[0:TC, 0:4, :, :], in_=v_d[:, 0:4])
        nc.gpsimd.dma_start(out=eb1[:], in_=eb_d[1])
        nc.scalar.dma_start(out=eb0[0:TC, 4:8, :, :], in_=eb_d[0, :, 4:8])
        nc.gpsimd.dma_start(out=vv[0:TC, 4:8, :, :], in_=v_d[:, 4:8])
        for ei in (3, 4):
            t = xt_pool.tile([128, 4, NH2], bf16, name=f"xte{ei}", tag="xt")
            nc.sync.dma_start(out=t[:], in_=xT_d[ei // 2, ei % 2])
            xts[ei] = t
        nc.scalar.dma_start(out=wp[:], in_=wp_d[:, :, :])
        bp = resid.tile([128, 4], f32, name="bp", tag="bp")
        nc.sync.dma_start(out=bp[:], in_=bp_d[:, :])
        eb = {0: eb0, 1: eb1}
        ones = resid.tile([128, 128], bf16, name="ones", tag="ones")
        nc.gpsimd.memset(ones[:], 1.0)

        # ACT table-set phase fence: chain every ACT instruction in emission
        # order (ordering-only edges) so the exp ops and gelu ops stay in
        # clean phases (different table sets, ~2.7us per reload).
        _prev_act = [None]

        def act(*args, **kwargs):
            inst = nc.scalar.activation(*args, **kwargs)
            if _prev_act[0] is not None:
                tile.add_dep_helper(inst.ins, _prev_act[0].ins, sync=False,
                                    reason="act order fence")
            _prev_act[0] = inst
            return inst

        def sc_mms(hp, qt, pst, ch):
            # Score matmuls for pair hp, chunk-half ch (chunks 2ch, 2ch+1).
            # The two heads sit at PE row groups 0/64 (tile_position), so the
            # two MMs of each chunk run concurrently.  PSUM tile layout
            # [128, head(2), inner(2), 512] -> each MM owns one bank.
            for ic in range(2):
                c = 2 * ch + ic
                for i in range(2):
                    nc.tensor.matmul(
                        pst[0:TC, i, ic, :],
                        lhsT=kT[i * 64:i * 64 + 64, hp, c * TC:(c + 1) * TC],
                        rhs=qt[hp][i * 64:i * 64 + 64, :],
                        tile_position=(i * 64, 0))

        def sc_exp(pst, ch, ees):
            # exp straight out of PSUM (bf16 out), one op per head per half:
            # [100, 2, 512] view = 2 banks.
            for i in range(2):
                act(ees[i][0:TC, 2 * ch:2 * ch + 2, :], pst[0:TC, i, :, :],
                    AF.Exp)

        def ee_mult(h, hf, ee):
            # ee *= exp(bias): bf16 SBUF->SBUF plain tensor_tensor (the only
            # DVE op class with a 2x_1P uop; scalar_tensor_tensor runs 1x),
            # in-place.  Applies the attention bias via the softmax identity
            # exp(s+b) = exp(s)*exp(b).
            e2 = ee[0:TC, :, :]
            ve = nc.vector
            ve.add_instruction(
                mybir.InstTensorTensor(
                    name=ve.bass.get_next_instruction_name(),
                    op=ALU.mult,
                    ins=[ve.lower_ap(e2),
                         ve.lower_ap(eb[hf][0:TC, h, :, :])],
                    outs=[ve.lower_ap(e2)],
                ))

        def denom_recip(h, ee):
            # Denominator for one head, self-broadcasting (see module doc).
            dps = ps_mm.tile([128, NH2], f32, name="dps", tag="psmm")
            for c in range(4):
                nc.tensor.matmul(
                    dps[:, :], lhsT=ones[0:TC, :], rhs=ee[0:TC, c, :],
                    start=(c == 0), stop=(c == 3))
            rbc = rbc_pool.tile([128, NH2], f32, name="rbc", tag="rbc")
            nc.vector.reciprocal_approx_fast(out=rbc[:], in_=dps[:])
            return rbc

        def av_head(h, ee, rbc, avn):
            # out^T = V^T @ ee for one head, then scale by 1/denom while
            # evacuating PSUM into the head's slice of the big avn tile.
            ps = ps_av.tile([128, 2, NH2], f32, name="psav", tag="psav")
            for dd in range(2):
                for tb in range(4):
                    nc.tensor.matmul(
                        ps[:, dd, :],
                        lhsT=vv[0:TC, h, tb, dd * 128:(dd + 1) * 128],
                        rhs=ee[0:TC, tb, :],
                        start=(tb == 0), stop=(tb == 3))
            for dd in range(2):
                nc.vector.scalar_tensor_tensor(
                    avn[:, 2 * h + dd, :], ps[:, dd, :], 0.0, rbc[:],
                    op0=ALU.bypass, op1=ALU.mult)

        def proj_m(pend, m):
            outg, fin = pend
            ps = ps_mm.tile([128, NH2], f32, name="psp", tag="psmm")
            for kc in range(16):
                nc.tensor.matmul(
                    ps[:],
                    lhsT=wp[:, kc, m * 128:(m + 1) * 128],
                    rhs=outg[:, kc, :],
                    start=(kc == 0), stop=(kc == 15))
            nc.vector.tensor_scalar(fin[:, m, :], ps[:], bp[:, m:m + 1], None,
                                    op0=ALU.add)

        def qproj_m(xt, m, qt_list):
            psq = ps_mm.tile([128, NH2], f32, name="psq", tag="psmm")
            for kc in range(4):
                nc.tensor.matmul(
                    psq[:],
                    lhsT=wq[:, kc, m * 128:(m + 1) * 128],
                    rhs=xt[:, kc, :],
                    start=(kc == 0), stop=(kc == 3))
            q = qt_pool.tile([128, NH2], bf16, name=f"qt{m}", tag="qt")
            nc.vector.tensor_scalar(q[:], psq[:], bq[:, m:m + 1], None,
                                    op0=ALU.add)
            qt_list.append(q)

        iters = [(b, hf) for b in range(B_LOC) for hf in range(2)]

        # Prologue: Q projection for iters 0 and 1 (the per-iter loop
        # computes qt two iterations ahead, so iteration 0 -- which has no
        # proj work to fill its softmax phase -- still has dense PE slots).
        qt = []
        for m in range(4):
            qproj_m(xts[0], m, qt)
        qt1 = []
        for m in range(4):
            qproj_m(xts[1], m, qt1)

        pend = None
        for it, (b, hf) in enumerate(iters):
            qt_next2 = []
            ees = {}        # head -> ee tile
            avn = avn_pool.tile([128, 16, NH2], bf16, name="avn", tag="avn")
            fin = fin_pool.tile([128, 4, NH2], bf16, name="fin", tag="fin")
            nxt = it + 2 < len(iters)
            for hp in range(4):
                pst = ps_sc.tile([128, 2, 2, NH2], f32, name="pst", tag="pssc")
                eep = []
                for i in range(2):
                    t = ee_pool.tile([TC, 4, NH2], bf16, name=f"ee{i}", tag="ee")
                    eep.append(t)
                    ees[2 * hp + i] = t
                # Slot 0 front-loads exp/gelu-independent PE work (the
                # iter+2 Q-proj rounds) so the PE has something to chew on
                # while the ACT drains the previous iter's gelu block and
                # reloads the exp table; proj rounds shift one slot later.
                sc_mms(hp, qt, pst, 0)
                if hp == 0:
                    if nxt:
                        qproj_m(xts[it + 2], 0, qt_next2)
                        qproj_m(xts[it + 2], 1, qt_next2)
                elif pend is not None:
                    proj_m(pend, hp - 1)
                sc_exp(pst, 0, eep)
                sc_mms(hp, qt, pst, 1)
                if hp == 0 and nxt:
                    qproj_m(xts[it + 2], 2, qt_next2)
                    qproj_m(xts[it + 2], 3, qt_next2)
                sc_exp(pst, 1, eep)
                for i in range(2):
                    ee_mult(2 * hp + i, hf, eep[i])
                if hp >= 1:
                    for i in range(2):
                        h = 2 * (hp - 1) + i
                        av_head(h, ees[h], denom_recip(h, ees[h]), avn)
            if pend is not None:
                proj_m(pend, 3)
                _, pf = pend
                pb, phf = iters[it - 1]
                nc.sync.dma_start(out=out_d[pb, phf], in_=pf[:])

            # GELU phase (own ACT table set), split around the last heads'
            # denominator/AV work so the PE bridges the ACT gelu block; proj
            # consumes outg slices [128, kc, :] directly.
            outg = outg_pool.tile([128, 16, NH2], bf16, name="og", tag="outg")
            for q4 in range(3):
                act(outg[:, 4 * q4:4 * q4 + 4, :], avn[:, 4 * q4:4 * q4 + 4, :],
                    AF.Gelu)
            av_head(6, ees[6], denom_recip(6, ees[6]), avn)
            act(outg[:, 12:14, :], avn[:, 12:14, :], AF.Gelu)
            av_head(7, ees[7], denom_recip(7, ees[7]), avn)
            act(outg[:, 14:16, :], avn[:, 14:16, :], AF.Gelu)
            pend = (outg, fin)

            qt = qt1
            qt1 = qt_next2
            if it + 5 < len(iters):
                bn, hfn = iters[it + 5]
                t = xt_pool.tile([128, 4, NH2], bf16, name="xt", tag="xt")
                nc.sync.dma_start(out=t[:], in_=xT_d[bn, hfn])
                xts[it + 5] = t

        # Epilogue: the final iteration's proj starts on the gelu slices
        # that are already done (kc 0..11) and finishes after the last gelu.
        outg, pf = pend
        pstE = ps_sc.tile([128, 2, 2, NH2], f32, name="pstE", tag="pssc")
        for m in range(4):
            ps = pstE[:, m // 2, m % 2, :]
            for kc in range(12):
                nc.tensor.matmul(ps, lhsT=wp[:, kc, m * 128:(m + 1) * 128],
                                 rhs=outg[:, kc, :],
                                 start=(kc == 0), stop=False)
        for m in range(4):
            ps = pstE[:, m // 2, m % 2, :]
            for kc in range(12, 16):
                nc.tensor.matmul(ps, lhsT=wp[:, kc, m * 128:(m + 1) * 128],
                                 rhs=outg[:, kc, :],
                                 start=False, stop=(kc == 15))
            nc.vector.tensor_scalar(pf[:, m, :], ps, bp[:, m:m + 1], None,
                                    op0=ALU.add)
        pb, phf = iters[-1]
        nc.sync.dma_start(out=out_d[pb, phf], in_=pf[:])

    nc.compile()
    return nc


def _prep_inputs(x, text, q_w, q_gamma, q_beta, q_mean, q_var,
                 kv_w, kv_gamma, kv_beta, kv_mean, kv_var,
                 proj_w, proj_gamma, proj_beta, proj_mean, proj_var,
                 attention_biases):
    """Host-side constant folding + layout prep. Returns per-core in_maps."""
    scale = KEY_DIM ** -0.5

    # Fold q BN + softmax scale into the q weight/bias.
    s_q = q_gamma / np.sqrt(q_var + EPS)
    wq_eff = (q_w * s_q[None, :] * scale).astype(np.float32)
    wq_pack = np.ascontiguousarray(
        wq_eff.reshape(4, 128, NH_KD).transpose(1, 0, 2)
    ).astype(ml_dtypes.bfloat16)
    bq_eff = ((q_beta - q_mean * s_q) * scale).astype(np.float32)
    bq_pack = np.ascontiguousarray(bq_eff.reshape(4, 128).T).astype(np.float32)

    # kv projection on host (shared across batch; ~1/150 of total FLOPs).
    s_kv = kv_gamma / np.sqrt(kv_var + EPS)
    kv = (text @ kv_w - kv_mean[None, :]) * s_kv[None, :] + kv_beta[None, :]
    kv = kv.astype(np.float32).reshape(NT, NUM_HEADS, KEY_DIM + D_V)
    k = kv[:, :, :KEY_DIM]          # (NT, H, KD)
    v = kv[:, :, KEY_DIM:]          # (NT, H, DV)
    # kT: [128 (pair-stacked kd), hp, NT]
    kT = k.transpose(1, 2, 0).reshape(4, 128, NT).transpose(1, 0, 2)
    kT = np.ascontiguousarray(kT).astype(ml_dtypes.bfloat16)
    # v: [t_local(100), h, chunk(4), dv]
    v_pack = np.ascontiguousarray(
        v.reshape(4, TC, NUM_HEADS, D_V).transpose(1, 2, 0, 3)
    ).astype(ml_dtypes.bfloat16)

    # exp() of the gathered relative position bias ->
    # [half, t_local(100), h, chunk(4), n(512)] bf16.
    n = np.arange(H_GRID * W_GRID)
    i, j = n // W_GRID, n % W_GRID
    t = np.arange(NT)
    a, bb = t // 100, t % 100
    idxs = np.abs(i[:, None] - a[None, :]) * 100 + np.abs(j[:, None] - bb[None, :])
    bias = attention_biases[:, idxs]                  # (H, N, NT) f32
    # (H, N, NT) -> [hf, t_local, h, chunk, n]
    biasT = bias.reshape(NUM_HEADS, 2, NH2, 4, TC).transpose(1, 4, 0, 3, 2)
    expbias = np.exp(np.ascontiguousarray(biasT)).astype(ml_dtypes.bfloat16)

    # Fold proj BN scale into wp, shift stays as epilogue bias.
    s_p = proj_gamma / np.sqrt(proj_var + EPS)
    wp_eff = (proj_w * s_p[None, :]).astype(np.float32)
    wp_pack = np.ascontiguousarray(
        wp_eff.reshape(16, 128, DIM).transpose(1, 0, 2)
    ).astype(ml_dtypes.bfloat16)
    bp_eff = (proj_beta - proj_mean * s_p).astype(np.float32)
    bp_pack = np.ascontiguousarray(bp_eff.reshape(4, 128).T).astype(np.float32)

    shared = {
        "wq": wq_pack, "bq": bq_pack, "kT": kT, "v": v_pack,
        "expbias": expbias, "wp": wp_pack, "bp": bp_pack,
    }
    in_maps = []
    for c in range(N_CORES):
        xs = x[c * B_LOC:(c + 1) * B_LOC]                       # (4, N, DIM)
        # [b, hf, p(128), kc(4), n(512)]
        xT = xs.transpose(0, 2, 1).reshape(B_LOC, 4, 128, 2, NH2)
        xT = np.ascontiguousarray(xT.transpose(0, 3, 2, 1, 4))
        m = dict(shared)
        m["xT"] = xT.astype(ml_dtypes.bfloat16)
        in_maps.append(m)
    return in_maps


def kernel(x, text, q_w, q_gamma, q_beta, q_mean, q_var,
           kv_w, kv_gamma, kv_beta, kv_mean, kv_var,
           proj_w, proj_gamma, proj_beta, proj_mean, proj_var,
           attention_biases, H, W, **_unused):
    from concourse.bass_utils import run_bass_kernel_spmd

    x = np.asarray(x, dtype=np.float32)
    in_maps = _prep_inputs(
        np.asarray(x, np.float32), np.asarray(text, np.float32),
        np.asarray(q_w, np.float32), np.asarray(q_gamma, np.float32),
        np.asarray(q_beta, np.float32), np.asarray(q_mean, np.float32),
        np.asarray(q_var, np.float32),
        np.asarray(kv_w, np.float32), np.asarray(kv_gamma, np.float32),
        np.asarray(kv_beta, np.float32), np.asarray(kv_mean, np.float32),
        np.asarray(kv_var, np.float32),
        np.asarray(proj_w, np.float32), np.asarray(proj_gamma, np.float32),
        np.asarray(proj_beta, np.float32), np.asarray(proj_mean, np.float32),
        np.asarray(proj_var, np.float32),
        np.asarray(attention_biases, np.float32))

    if "nc" not in _CACHE:
        _CACHE["nc"] = _build_nc()
    nc = _CACHE["nc"]

    res = run_bass_kernel_spmd(nc, in_maps, list(range(N_CORES)))
    outs = [np.asarray(res.results[c]["outT"], dtype=np.float32)
            for c in range(N_CORES)]               # (4, 2, 128, 4, 512)
    full = np.concatenate(outs, axis=0)            # (B, 2, 128, 4, 512)
    # out[b, hf, p, m, n] = final[dim=m*128+p, tok=hf*512+n]
    full = full.transpose(0, 1, 4, 3, 2).reshape(B, N_TOK, DIM)
    return np.ascontiguousarray(full)


# revision 36
# speedup vs baseline: 1.0956x; 1.0956x over previous
"""Trainium2 Bass kernel for LeViT-style cross attention (nn_Attention).

Strategy: pure data-parallel over batch B=32 across 8 NeuronCores (4 per
core, no collectives).  Host precomputes the tiny shared pieces (BN folds,
the 400x2560 kv projection, exp() of the gathered relative-position bias
table) and pre-transposes layouts; each core runs the heavy ~19 GFLOP of
per-batch attention: Q projection, scores+softmax, attn@V, GELU, output
projection.

Device kernel per core, per (batch, n-half of 512), all layouts transposed
(feature dim on partitions) so no on-chip transposes are needed:
  A: Q^T = Wq^T @ X^T             (bf16 matmul, bias epilogue on DVE)
  B per head pair (heads packed at PE row groups 0/64 so the K=64 score
     matmuls row-tile the PE array and run concurrently):
       scoresT[t,n] accumulate into a 4-bank PSUM tile
       ee = exp(scoresT) (ACT, reads PSUM directly, bf16 out)
       ee *= exp(bias)   (DVE bf16 2x tensor_tensor, in-place; softmax
                          identity exp(s+b) = exp(s)*exp(b) keeps the bias
                          add off the PSUM-source fp32 1x DVE path)
     per head:
       denominator self-broadcasting: lhsT = ones[100,128] makes the PE
       write the row-sum to ALL 128 psum partitions, so the reciprocal
       (custom-DVE reciprocal_approx_fast) comes out already broadcast
       (partition_broadcast from a non-zero base partition reads the wrong
       row on HW -- probe-verified)
       out^T = V^T @ ee (bf16) -> PSUM; avn = psum * recip (DVE)
  C: gelu on one contiguous avn tile (big ACT ops, split around the last
     heads' AV so the PE bridges the gelu table-switch block), then
     proj: final^T = Wp^T @ gelu^T + folded-BN bias epilogue, DMA out
     bf16 as final^T; host transposes back / casts.

All resident weights are host-packed contiguous-per-partition so each loads
with ONE DMA (each DMA instruction costs ~800ns of serial sync-queue issue
time; 75 small DMAs was a ~50us startup stall).  Q-projection runs two
iterations ahead so iteration 0 (which has no proj work) still has dense PE
slots.
"""

import numpy as np
import ml_dtypes

# Model hyperparameters (hardcoded per spec nn_Attention_81449759801699)
B, N_TOK, DIM = 32, 1024, 512
NT = 400
NUM_HEADS, KEY_DIM = 8, 64
D_V = 256
DH = D_V * NUM_HEADS          # 2048
NH_KD = KEY_DIM * NUM_HEADS   # 512
H_KV = DH + NH_KD             # 2560
H_GRID, W_GRID = 32, 32
EPS = 1e-5
N_CORES = 8
B_LOC = B // N_CORES          # 4 batches per core
NH2 = 512                     # n-half
TC = 100                      # t-chunk (400 = 4*100)

_CACHE = {}


def _build_nc():
    """Build + compile the single-core Bass graph (same graph on all 8 cores)."""
    from contextlib import ExitStack
    import concourse.bass as bass
    import concourse.bacc as bacc
    import concourse.tile as tile
    from concourse import mybir

    f32 = mybir.dt.float32
    bf16 = mybir.dt.bfloat16
    AF = mybir.ActivationFunctionType
    ALU = mybir.AluOpType

    nc = bacc.Bacc("TRN2", target_bir_lowering=False, debug=False,
                   num_devices=N_CORES)

    # All tensors host-packed [128-partition dim, contiguous free] so every
    # load is a single DMA.
    xT_d = nc.dram_tensor("xT", [B_LOC, 2, 128, 4, NH2], bf16,
                          kind="ExternalInput")
    wq_d = nc.dram_tensor("wq", [128, 4, NH_KD], bf16, kind="ExternalInput")
    bq_d = nc.dram_tensor("bq", [128, 4], f32, kind="ExternalInput")
    kT_d = nc.dram_tensor("kT", [128, NUM_HEADS // 2, NT], bf16,
                          kind="ExternalInput")
    v_d = nc.dram_tensor("v", [TC, NUM_HEADS, 4, D_V], bf16,
                         kind="ExternalInput")
    # exp(bias): [half, t_local(100), h, chunk(4), n(512)]
    eb_d = nc.dram_tensor("expbias", [2, TC, NUM_HEADS, 4, NH2], bf16,
                          kind="ExternalInput")
    wp_d = nc.dram_tensor("wp", [128, 16, DIM], bf16, kind="ExternalInput")
    bp_d = nc.dram_tensor("bp", [128, 4], f32, kind="ExternalInput")
    out_d = nc.dram_tensor("outT", [B_LOC, 2, 128, 4, NH2], bf16,
                           kind="ExternalOutput")

    with tile.TileContext(nc) as tc, ExitStack() as ctx:
        resid = ctx.enter_context(tc.tile_pool(name="resid", bufs=1))
        xt_pool = ctx.enter_context(tc.tile_pool(name="xt", bufs=4))
        qt_pool = ctx.enter_context(tc.tile_pool(name="qt", bufs=12))
        ee_pool = ctx.enter_context(tc.tile_pool(name="ep", bufs=4))
        rbc_pool = ctx.enter_context(tc.tile_pool(name="rbcp", bufs=2))
        avn_pool = ctx.enter_context(tc.tile_pool(name="avnp", bufs=1))
        outg_pool = ctx.enter_context(tc.tile_pool(name="outgp", bufs=2))
        fin_pool = ctx.enter_context(tc.tile_pool(name="finp", bufs=2))
        ps_sc = ctx.enter_context(tc.tile_pool(name="pssc", bufs=1, space="PSUM"))
        ps_av = ctx.enter_context(tc.tile_pool(name="psav", bufs=1, space="PSUM"))
        # Qproj, proj and denominator rounds share one 2-slot bank pool so
        # consecutive rounds pipeline instead of serializing on the evac.
        ps_mm = ctx.enter_context(tc.tile_pool(name="psmm", bufs=2, space="PSUM"))

        # ---- resident weights (one DMA each, ordered by first use) ----
        wq = resid.tile([128, 4, NH_KD], bf16, name="wq", tag="wq")
        nc.sync.dma_start(out=wq[:], in_=wq_d[:, :, :])
        bq = resid.tile([128, 4], f32, name="bq", tag="bq")
        nc.sync.dma_start(out=bq[:], in_=bq_d[:, :])
        kT = resid.tile([128, NUM_HEADS // 2, NT], bf16, name="kT", tag="kT")
        nc.sync.dma_start(out=kT[:], in_=kT_d[:, :, :])

        # First three iterations' x tiles, ahead of the bulky residents.
        xts = {}
        for ei in range(3):
            t = xt_pool.tile([128, 4, NH2], bf16, name=f"xte{ei}", tag="xt")
            nc.sync.dma_start(out=t[:], in_=xT_d[ei // 2, ei % 2])
            xts[ei] = t

        # eb/v interleaved in first-use order (heads 0-3 of half 0 are
        # needed within ~15us of kernel start); x for iters 3-4 and the
        # hf=1 exp-bias table land before wp (first used an iteration
        # later) -- the qproj-two-ahead schedule consumes x(it+2) early.
        # eb/v interleaved in first-use order (heads 0-3 of half 0 are
        # needed within ~15us of kernel start); x for iters 3-4 and the
        # hf=1 exp-bias table land before wp (first used an iteration
        # later) -- the qproj-two-ahead schedule consumes x(it+2) early.
        eb0 = resid.tile([TC, NUM_HEADS, 4, NH2], bf16, name="eb0", tag="eb0")
        vv = resid.tile([TC, NUM_HEADS, 4, D_V], bf16, name="vv", tag="vv")
        nc.sync.dma_start(out=eb0[0:TC, 0:4, :, :], in_=eb_d[0, :, 0:4])
        nc.sync.dma_start(out=vv[0:TC, 0:4, :, :], in_=v_d[:, 0:4])
        nc.sync.dma_start(out=eb0[0:TC, 4:8, :, :], in_=eb_d[0, :, 4:8])
        nc.sync.dma_start(out=vv[0:TC, 4:8, :, :], in_=v_d[:, 4:8])
        for ei in (3, 4):
            t = xt_pool.tile([128, 4, NH2], bf16, name=f"xte{ei}", tag="xt")
            nc.sync.dma_start(out=t[:], in_=xT_d[ei // 2, ei % 2])
            xts[ei] = t
        eb1 = resid.tile([TC, NUM_HEADS, 4, NH2], bf16, name="eb1", tag="eb1")
        nc.sync.dma_start(out=eb1[:], in_=eb_d[1])
        wp = resid.tile([128, 16, DIM], bf16, name="wp", tag="wp")
        nc.sync.dma_start(out=wp[:], in_=wp_d[:, :, :])
        bp = resid.tile([128, 4], f32, name="bp", tag="bp")
        nc.sync.dma_start(out=bp[:], in_=bp_d[:, :])
        eb = {0: eb0, 1: eb1}
        ones = resid.tile([128, 128], bf16, name="ones", tag="ones")
        nc.gpsimd.memset(ones[:], 1.0)

        # ACT table-set phase fence: chain every ACT instruction in emission
        # order (ordering-only edges) so the exp ops and gelu ops stay in
        # clean phases (different table sets, ~2.7us per reload).
        _prev_act = [None]

        def act(*args, **kwargs):
            inst = nc.scalar.activation(*args, **kwargs)
            if _prev_act[0] is not None:
                tile.add_dep_helper(inst.ins, _prev_act[0].ins, sync=False,
                                    reason="act order fence")
            _prev_act[0] = inst
            return inst

        def sc_mms(hp, qt, pst, ch):
            # Score matmuls for pair hp, chunk-half ch (chunks 2ch, 2ch+1).
            # The two heads sit at PE row groups 0/64 (tile_position), so the
            # two MMs of each chunk run concurrently.  PSUM tile layout
            # [128, head(2), inner(2), 512] -> each MM owns one bank.
            for ic in range(2):
                c = 2 * ch + ic
                for i in range(2):
                    nc.tensor.matmul(
                        pst[0:TC, i, ic, :],
                        lhsT=kT[i * 64:i * 64 + 64, hp, c * TC:(c + 1) * TC],
                        rhs=qt[hp][i * 64:i * 64 + 64, :],
                        tile_position=(i * 64, 0))

        def sc_exp(pst, ch, ees):
            # exp straight out of PSUM (bf16 out), one op per head per half:
            # [100, 2, 512] view = 2 banks.
            for i in range(2):
                act(ees[i][0:TC, 2 * ch:2 * ch + 2, :], pst[0:TC, i, :, :],
                    AF.Exp)

        def ee_mult(h, hf, ee):
            # ee *= exp(bias): bf16 SBUF->SBUF plain tensor_tensor (the only
            # DVE op class with a 2x_1P uop; scalar_tensor_tensor runs 1x),
            # in-place.  Applies the attention bias via the softmax identity
            # exp(s+b) = exp(s)*exp(b).
            e2 = ee[0:TC, :, :]
            ve = nc.vector
            ve.add_instruction(
                mybir.InstTensorTensor(
                    name=ve.bass.get_next_instruction_name(),
                    op=ALU.mult,
                    ins=[ve.lower_ap(e2),
                         ve.lower_ap(eb[hf][0:TC, h, :, :])],
                    outs=[ve.lower_ap(e2)],
                ))

        def denom_recip(h, ee):
            # Denominator for one head, self-broadcasting (see module doc).
            dps = ps_mm.tile([128, NH2], f32, name="dps", tag="psmm")
            for c in range(4):
                nc.tensor.matmul(
                    dps[:, :], lhsT=ones[0:TC, :], rhs=ee[0:TC, c, :],
                    start=(c == 0), stop=(c == 3))
            rbc = rbc_pool.tile([128, NH2], f32, name="rbc", tag="rbc")
            nc.vector.reciprocal_approx_fast(out=rbc[:], in_=dps[:])
            return rbc

        def av_head(h, ee, rbc, avn):
            # out^T = V^T @ ee for one head, then scale by 1/denom while
            # evacuating PSUM into the head's slice of the big avn tile.
            ps = ps_av.tile([128, 2, NH2], f32, name="psav", tag="psav")
            for dd in range(2):
                for tb in range(4):
                    nc.tensor.matmul(
                        ps[:, dd, :],
                        lhsT=vv[0:TC, h, tb, dd * 128:(dd + 1) * 128],
                        rhs=ee[0:TC, tb, :],
                        start=(tb == 0), stop=(tb == 3))
            for dd in range(2):
                nc.vector.scalar_tensor_tensor(
                    avn[:, 2 * h + dd, :], ps[:, dd, :], 0.0, rbc[:],
                    op0=ALU.bypass, op1=ALU.mult)

        def proj_m(pend, m):
            outg, fin = pend
            ps = ps_mm.tile([128, NH2], f32, name="psp", tag="psmm")
            for kc in range(16):
                nc.tensor.matmul(
                    ps[:],
                    lhsT=wp[:, kc, m * 128:(m + 1) * 128],
                    rhs=outg[:, kc, :],
                    start=(kc == 0), stop=(kc == 15))
            nc.vector.tensor_scalar(fin[:, m, :], ps[:], bp[:, m:m + 1], None,
                                    op0=ALU.add)

        def qproj_m(xt, m, qt_list):
            psq = ps_mm.tile([128, NH2], f32, name="psq", tag="psmm")
            for kc in range(4):
                nc.tensor.matmul(
                    psq[:],
                    lhsT=wq[:, kc, m * 128:(m + 1) * 128],
                    rhs=xt[:, kc, :],
                    start=(kc == 0), stop=(kc == 3))
            q = qt_pool.tile([128, NH2], bf16, name=f"qt{m}", tag="qt")
            nc.vector.tensor_scalar(q[:], psq[:], bq[:, m:m + 1], None,
                                    op0=ALU.add)
            qt_list.append(q)

        iters = [(b, hf) for b in range(B_LOC) for hf in range(2)]

        # Prologue: Q projection for iters 0 and 1 (the per-iter loop
        # computes qt two iterations ahead, so iteration 0 -- which has no
        # proj work to fill its softmax phase -- still has dense PE slots).
        qt = []
        for m in range(4):
            qproj_m(xts[0], m, qt)
        qt1 = []
        for m in range(4):
            qproj_m(xts[1], m, qt1)

        pend = None
        for it, (b, hf) in enumerate(iters):
            qt_next2 = []
            ees = {}        # head -> ee tile
            avn = avn_pool.tile([128, 16, NH2], bf16, name="avn", tag="avn")
            fin = fin_pool.tile([128, 4, NH2], bf16, name="fin", tag="fin")
            nxt = it + 2 < len(iters)
            for hp in range(4):
                pst = ps_sc.tile([128, 2, 2, NH2], f32, name="pst", tag="pssc")
                eep = []
                for i in range(2):
                    t = ee_pool.tile([TC, 4, NH2], bf16, name=f"ee{i}", tag="ee")
                    eep.append(t)
                    ees[2 * hp + i] = t
                # Slot 0 front-loads exp/gelu-independent PE work (the
                # iter+2 Q-proj rounds) so the PE has something to chew on
                # while the ACT drains the previous iter's gelu block and
                # reloads the exp table; proj rounds shift one slot later.
                sc_mms(hp, qt, pst, 0)
                if hp == 0:
                    if nxt:
                        qproj_m(xts[it + 2], 0, qt_next2)
                        qproj_m(xts[it + 2], 1, qt_next2)
                elif pend is not None:
                    proj_m(pend, hp - 1)
                sc_exp(pst, 0, eep)
                sc_mms(hp, qt, pst, 1)
                if hp == 0 and nxt:
                    qproj_m(xts[it + 2], 2, qt_next2)
                    qproj_m(xts[it + 2], 3, qt_next2)
                sc_exp(pst, 1, eep)
                for i in range(2):
                    ee_mult(2 * hp + i, hf, eep[i])
                if hp >= 1:
                    for i in range(2):
                        h = 2 * (hp - 1) + i
                        av_head(h, ees[h], denom_recip(h, ees[h]), avn)
            if pend is not None:
                proj_m(pend, 3)
                _, pf = pend
                pb, phf = iters[it - 1]
                nc.sync.dma_start(out=out_d[pb, phf], in_=pf[:])

            # GELU phase (own ACT table set), split around the last heads'
            # denominator/AV work so the PE bridges the ACT gelu block; proj
            # consumes outg slices [128, kc, :] directly.
            outg = outg_pool.tile([128, 16, NH2], bf16, name="og", tag="outg")
            for q4 in range(3):
                act(outg[:, 4 * q4:4 * q4 + 4, :], avn[:, 4 * q4:4 * q4 + 4, :],
                    AF.Gelu)
            av_head(6, ees[6], denom_recip(6, ees[6]), avn)
            act(outg[:, 12:14, :], avn[:, 12:14, :], AF.Gelu)
            av_head(7, ees[7], denom_recip(7, ees[7]), avn)
            act(outg[:, 14:16, :], avn[:, 14:16, :], AF.Gelu)
            pend = (outg, fin)

            qt = qt1
            qt1 = qt_next2
            if it + 5 < len(iters):
                bn, hfn = iters[it + 5]
                t = xt_pool.tile([128, 4, NH2], bf16, name="xt", tag="xt")
                nc.sync.dma_start(out=t[:], in_=xT_d[bn, hfn])
                xts[it + 5] = t

        # Epilogue: the final iteration's proj starts on the gelu slices
        # that are already done (kc 0..11) and finishes after the last gelu.
        outg, pf = pend
        pstE = ps_sc.tile([128, 2, 2, NH2], f32, name="pstE", tag="pssc")
        for m in range(4):
            ps = pstE[:, m // 2, m % 2, :]
            for kc in range(12):
                nc.tensor.matmul(ps, lhsT=wp[:, kc, m * 128:(m + 1) * 128],
                                 rhs=outg[:, kc, :],
                                 start=(kc == 0), stop=False)
        for m in range(4):
            ps = pstE[:, m // 2, m % 2, :]
            for kc in range(12, 16):
                nc.tensor.matmul(ps, lhsT=wp[:, kc, m * 128:(m + 1) * 128],
                                 rhs=outg[:, kc, :],
                                 start=False, stop=(kc == 15))
            nc.vector.tensor_scalar(pf[:, m, :], ps, bp[:, m:m + 1], None,
                                    op0=ALU.add)
        pb, phf = iters[-1]
        nc.sync.dma_start(out=out_d[pb, phf], in_=pf[:])

    nc.compile()
    return nc


def _prep_inputs(x, text, q_w, q_gamma, q_beta, q_mean, q_var,
                 kv_w, kv_gamma, kv_beta, kv_mean, kv_var,
                 proj_w, proj_gamma, proj_beta, proj_mean, proj_var,
                 attention_biases):
    """Host-side constant folding + layout prep. Returns per-core in_maps."""
    scale = KEY_DIM ** -0.5

    # Fold q BN + softmax scale into the q weight/bias.
    s_q = q_gamma / np.sqrt(q_var + EPS)
    wq_eff = (q_w * s_q[None, :] * scale).astype(np.float32)
    wq_pack = np.ascontiguousarray(
        wq_eff.reshape(4, 128, NH_KD).transpose(1, 0, 2)
    ).astype(ml_dtypes.bfloat16)
    bq_eff = ((q_beta - q_mean * s_q) * scale).astype(np.float32)
    bq_pack = np.ascontiguousarray(bq_eff.reshape(4, 128).T).astype(np.float32)

    # kv projection on host (shared across batch; ~1/150 of total FLOPs).
    s_kv = kv_gamma / np.sqrt(kv_var + EPS)
    kv = (text @ kv_w - kv_mean[None, :]) * s_kv[None, :] + kv_beta[None, :]
    kv = kv.astype(np.float32).reshape(NT, NUM_HEADS, KEY_DIM + D_V)
    k = kv[:, :, :KEY_DIM]          # (NT, H, KD)
    v = kv[:, :, KEY_DIM:]          # (NT, H, DV)
    # kT: [128 (pair-stacked kd), hp, NT]
    kT = k.transpose(1, 2, 0).reshape(4, 128, NT).transpose(1, 0, 2)
    kT = np.ascontiguousarray(kT).astype(ml_dtypes.bfloat16)
    # v: [t_local(100), h, chunk(4), dv]
    v_pack = np.ascontiguousarray(
        v.reshape(4, TC, NUM_HEADS, D_V).transpose(1, 2, 0, 3)
    ).astype(ml_dtypes.bfloat16)

    # exp() of the gathered relative position bias ->
    # [half, t_local(100), h, chunk(4), n(512)] bf16.
    n = np.arange(H_GRID * W_GRID)
    i, j = n // W_GRID, n % W_GRID
    t = np.arange(NT)
    a, bb = t // 100, t % 100
    idxs = np.abs(i[:, None] - a[None, :]) * 100 + np.abs(j[:, None] - bb[None, :])
    bias = attention_biases[:, idxs]                  # (H, N, NT) f32
    # (H, N, NT) -> [hf, t_local, h, chunk, n]
    biasT = bias.reshape(NUM_HEADS, 2, NH2, 4, TC).transpose(1, 4, 0, 3, 2)
    expbias = np.exp(np.ascontiguousarray(biasT)).astype(ml_dtypes.bfloat16)

    # Fold proj BN scale into wp, shift stays as epilogue bias.
    s_p = proj_gamma / np.sqrt(proj_var + EPS)
    wp_eff = (proj_w * s_p[None, :]).astype(np.float32)
    wp_pack = np.ascontiguousarray(
        wp_eff.reshape(16, 128, DIM).transpose(1, 0, 2)
    ).astype(ml_dtypes.bfloat16)
    bp_eff = (proj_beta - proj_mean * s_p).astype(np.float32)
    bp_pack = np.ascontiguousarray(bp_eff.reshape(4, 128).T).astype(np.float32)

    shared = {
        "wq": wq_pack, "bq": bq_pack, "kT": kT, "v": v_pack,
        "expbias": expbias, "wp": wp_pack, "bp": bp_pack,
    }
    in_maps = []
    for c in range(N_CORES):
        xs = x[c * B_LOC:(c + 1) * B_LOC]                       # (4, N, DIM)
        # [b, hf, p(128), kc(4), n(512)]
        xT = xs.transpose(0, 2, 1).reshape(B_LOC, 4, 128, 2, NH2)
        xT = np.ascontiguousarray(xT.transpose(0, 3, 2, 1, 4))
        m = dict(shared)
        m["xT"] = xT.astype(ml_dtypes.bfloat16)
        in_maps.append(m)
    return in_maps


def kernel(x, text, q_w, q_gamma, q_beta, q_mean, q_var,
           kv_w, kv_gamma, kv_beta, kv_mean, kv_var,
           proj_w, proj_gamma, proj_beta, proj_mean, proj_var,
           attention_biases, H, W, **_unused):
    from concourse.bass_utils import run_bass_kernel_spmd

    x = np.asarray(x, dtype=np.float32)
    in_maps = _prep_inputs(
        np.asarray(x, np.float32), np.asarray(text, np.float32),
        np.asarray(q_w, np.float32), np.asarray(q_gamma, np.float32),
        np.asarray(q_beta, np.float32), np.asarray(q_mean, np.float32),
        np.asarray(q_var, np.float32),
        np.asarray(kv_w, np.float32), np.asarray(kv_gamma, np.float32),
        np.asarray(kv_beta, np.float32), np.asarray(kv_mean, np.float32),
        np.asarray(kv_var, np.float32),
        np.asarray(proj_w, np.float32), np.asarray(proj_gamma, np.float32),
        np.asarray(proj_beta, np.float32), np.asarray(proj_mean, np.float32),
        np.asarray(proj_var, np.float32),
        np.asarray(attention_biases, np.float32))

    if "nc" not in _CACHE:
        _CACHE["nc"] = _build_nc()
    nc = _CACHE["nc"]

    res = run_bass_kernel_spmd(nc, in_maps, list(range(N_CORES)))
    outs = [np.asarray(res.results[c]["outT"], dtype=np.float32)
            for c in range(N_CORES)]               # (4, 2, 128, 4, 512)
    full = np.concatenate(outs, axis=0)            # (B, 2, 128, 4, 512)
    # out[b, hf, p, m, n] = final[dim=m*128+p, tok=hf*512+n]
    full = full.transpose(0, 1, 4, 3, 2).reshape(B, N_TOK, DIM)
    return np.ascontiguousarray(full)


# revision 38
# speedup vs baseline: 1.1009x; 1.0048x over previous
"""Trainium2 Bass kernel for LeViT-style cross attention (nn_Attention).

Strategy: pure data-parallel over batch B=32 across 8 NeuronCores (4 per
core, no collectives).  Host precomputes the tiny shared pieces (BN folds,
the 400x2560 kv projection, exp() of the gathered relative-position bias
table) and pre-transposes layouts; each core runs the heavy ~19 GFLOP of
per-batch attention: Q projection, scores+softmax, attn@V, GELU, output
projection.

Device kernel per core, per (batch, n-half of 512), all layouts transposed
(feature dim on partitions) so no on-chip transposes are needed:
  A: Q^T = Wq^T @ X^T             (bf16 matmul, bias epilogue on DVE)
  B per head pair (heads packed at PE row groups 0/64 so the K=64 score
     matmuls row-tile the PE array and run concurrently):
       scoresT[t,n] accumulate into a 4-bank PSUM tile
       ee = exp(scoresT) (ACT, reads PSUM directly, bf16 out)
       ee *= exp(bias)   (DVE bf16 2x tensor_tensor, in-place; softmax
                          identity exp(s+b) = exp(s)*exp(b) keeps the bias
                          add off the PSUM-source fp32 1x DVE path)
     per head:
       denominator self-broadcasting: lhsT = ones[100,128] makes the PE
       write the row-sum to ALL 128 psum partitions, so the reciprocal
       (custom-DVE reciprocal_approx_fast) comes out already broadcast
       (partition_broadcast from a non-zero base partition reads the wrong
       row on HW -- probe-verified)
       out^T = V^T @ ee (bf16) -> PSUM; avn = psum * recip (DVE)
  C: gelu on one contiguous avn tile (big ACT ops, split around the last
     heads' AV so the PE bridges the gelu table-switch block), then
     proj: final^T = Wp^T @ gelu^T + folded-BN bias epilogue, DMA out
     bf16 as final^T; host transposes back / casts.

All resident weights are host-packed contiguous-per-partition so each loads
with ONE DMA (each DMA instruction costs ~800ns of serial sync-queue issue
time; 75 small DMAs was a ~50us startup stall).  Q-projection runs two
iterations ahead so iteration 0 (which has no proj work) still has dense PE
slots.
"""

import numpy as np
import ml_dtypes

# Model hyperparameters (hardcoded per spec nn_Attention_81449759801699)
B, N_TOK, DIM = 32, 1024, 512
NT = 400
NUM_HEADS, KEY_DIM = 8, 64
D_V = 256
DH = D_V * NUM_HEADS          # 2048
NH_KD = KEY_DIM * NUM_HEADS   # 512
H_KV = DH + NH_KD             # 2560
H_GRID, W_GRID = 32, 32
EPS = 1e-5
N_CORES = 8
B_LOC = B // N_CORES          # 4 batches per core
NH2 = 512                     # n-half
TC = 100                      # t-chunk (400 = 4*100)

_CACHE = {}


def _build_nc():
    """Build + compile the single-core Bass graph (same graph on all 8 cores)."""
    from contextlib import ExitStack
    import concourse.bass as bass
    import concourse.bacc as bacc
    import concourse.tile as tile
    from concourse import mybir

    f32 = mybir.dt.float32
    bf16 = mybir.dt.bfloat16
    AF = mybir.ActivationFunctionType
    ALU = mybir.AluOpType

    nc = bacc.Bacc("TRN2", target_bir_lowering=False, debug=False,
                   num_devices=N_CORES)

    # All tensors host-packed [128-partition dim, contiguous free] so every
    # load is a single DMA.
    xT_d = nc.dram_tensor("xT", [B_LOC, 2, 128, 4, NH2], bf16,
                          kind="ExternalInput")
    wq_d = nc.dram_tensor("wq", [128, 4, NH_KD], bf16, kind="ExternalInput")
    bq_d = nc.dram_tensor("bq", [128, 4], f32, kind="ExternalInput")
    kT_d = nc.dram_tensor("kT", [128, NUM_HEADS // 2, NT], bf16,
                          kind="ExternalInput")
    v_d = nc.dram_tensor("v", [TC, NUM_HEADS, 4, D_V], bf16,
                         kind="ExternalInput")
    # exp(bias): [half, t_local(100), h, chunk(4), n(512)]
    eb_d = nc.dram_tensor("expbias", [2, TC, NUM_HEADS, 4, NH2], bf16,
                          kind="ExternalInput")
    wp_d = nc.dram_tensor("wp", [128, 16, DIM], bf16, kind="ExternalInput")
    bp_d = nc.dram_tensor("bp", [128, 4], f32, kind="ExternalInput")
    out_d = nc.dram_tensor("outT", [B_LOC, 2, 128, 4, NH2], bf16,
                           kind="ExternalOutput")

    with tile.TileContext(nc) as tc, ExitStack() as ctx:
        resid = ctx.enter_context(tc.tile_pool(name="resid", bufs=1))
        xt_pool = ctx.enter_context(tc.tile_pool(name="xt", bufs=4))
        qt_pool = ctx.enter_context(tc.tile_pool(name="qt", bufs=12))
        ee_pool = ctx.enter_context(tc.tile_pool(name="ep", bufs=4))
        rbc_pool = ctx.enter_context(tc.tile_pool(name="rbcp", bufs=2))
        avn_pool = ctx.enter_context(tc.tile_pool(name="avnp", bufs=1))
        outg_pool = ctx.enter_context(tc.tile_pool(name="outgp", bufs=2))
        fin_pool = ctx.enter_context(tc.tile_pool(name="finp", bufs=2))
        ps_sc = ctx.enter_context(tc.tile_pool(name="pssc", bufs=1, space="PSUM"))
        ps_av = ctx.enter_context(tc.tile_pool(name="psav", bufs=1, space="PSUM"))
        # Qproj, proj and denominator rounds share one 2-slot bank pool so
        # consecutive rounds pipeline instead of serializing on the evac.
        ps_mm = ctx.enter_context(tc.tile_pool(name="psmm", bufs=2, space="PSUM"))

        # ---- resident weights (one DMA each, ordered by first use) ----
        wq = resid.tile([128, 4, NH_KD], bf16, name="wq", tag="wq")
        nc.sync.dma_start(out=wq[:], in_=wq_d[:, :, :])
        bq = resid.tile([128, 4], f32, name="bq", tag="bq")
        nc.sync.dma_start(out=bq[:], in_=bq_d[:, :])
        kT = resid.tile([128, NUM_HEADS // 2, NT], bf16, name="kT", tag="kT")
        nc.sync.dma_start(out=kT[:], in_=kT_d[:, :, :])

        # First three iterations' x tiles, ahead of the bulky residents.
        xts = {}
        for ei in range(3):
            t = xt_pool.tile([128, 4, NH2], bf16, name=f"xte{ei}", tag="xt")
            nc.sync.dma_start(out=t[:], in_=xT_d[ei // 2, ei % 2])
            xts[ei] = t

        # eb/v interleaved in first-use order (heads 0-3 of half 0 are
        # needed within ~15us of kernel start); x for iters 3-4 and the
        # hf=1 exp-bias table land before wp (first used an iteration
        # later) -- the qproj-two-ahead schedule consumes x(it+2) early.
        eb0 = resid.tile([TC, NUM_HEADS, 4, NH2], bf16, name="eb0", tag="eb0")
        vv = resid.tile([TC, NUM_HEADS, 4, D_V], bf16, name="vv", tag="vv")
        nc.sync.dma_start(out=eb0[0:TC, 0:4, :, :], in_=eb_d[0, :, 0:4])
        nc.sync.dma_start(out=vv[0:TC, 0:4, :, :], in_=v_d[:, 0:4])
        nc.sync.dma_start(out=eb0[0:TC, 4:8, :, :], in_=eb_d[0, :, 4:8])
        nc.sync.dma_start(out=vv[0:TC, 4:8, :, :], in_=v_d[:, 4:8])
        for ei in (3, 4):
            t = xt_pool.tile([128, 4, NH2], bf16, name=f"xte{ei}", tag="xt")
            nc.sync.dma_start(out=t[:], in_=xT_d[ei // 2, ei % 2])
            xts[ei] = t
        eb1 = resid.tile([TC, NUM_HEADS, 4, NH2], bf16, name="eb1", tag="eb1")
        nc.sync.dma_start(out=eb1[:], in_=eb_d[1])
        wp = resid.tile([128, 16, DIM], bf16, name="wp", tag="wp")
        nc.sync.dma_start(out=wp[:], in_=wp_d[:, :, :])
        bp = resid.tile([128, 4], f32, name="bp", tag="bp")
        nc.sync.dma_start(out=bp[:], in_=bp_d[:, :])
        eb = {0: eb0, 1: eb1}
        ones = resid.tile([128, 128], bf16, name="ones", tag="ones")
        nc.gpsimd.memset(ones[:], 1.0)

        # ACT table-set phase fence: chain every ACT instruction in emission
        # order (ordering-only edges) so the exp ops and gelu ops stay in
        # clean phases (different table sets, ~2.7us per reload).
        _prev_act = [None]

        def act(*args, **kwargs):
            inst = nc.scalar.activation(*args, **kwargs)
            if _prev_act[0] is not None:
                tile.add_dep_helper(inst.ins, _prev_act[0].ins, sync=False,
                                    reason="act order fence")
            _prev_act[0] = inst
            return inst

        def sc_mms(hp, qt, pst, ch):
            # Score matmuls for pair hp, chunk-half ch (chunks 2ch, 2ch+1).
            # The two heads sit at PE row groups 0/64 (tile_position), so the
            # two MMs of each chunk run concurrently.  PSUM tile layout
            # [128, head(2), inner(2), 512] -> each MM owns one bank.
            for ic in range(2):
                c = 2 * ch + ic
                for i in range(2):
                    nc.tensor.matmul(
                        pst[0:TC, i, ic, :],
                        lhsT=kT[i * 64:i * 64 + 64, hp, c * TC:(c + 1) * TC],
                        rhs=qt[hp][i * 64:i * 64 + 64, :],
                        tile_position=(i * 64, 0))

        def sc_exp(pst, ch, ees):
            # exp straight out of PSUM (bf16 out), one op per head per half:
            # [100, 2, 512] view = 2 banks.
            for i in range(2):
                act(ees[i][0:TC, 2 * ch:2 * ch + 2, :], pst[0:TC, i, :, :],
                    AF.Exp)

        def ee_mult(h, hf, ee):
            # ee *= exp(bias): bf16 SBUF->SBUF plain tensor_tensor (the only
            # DVE op class with a 2x_1P uop; scalar_tensor_tensor runs 1x),
            # in-place.  Applies the attention bias via the softmax identity
            # exp(s+b) = exp(s)*exp(b).
            e2 = ee[0:TC, :, :]
            ve = nc.vector
            ve.add_instruction(
                mybir.InstTensorTensor(
                    name=ve.bass.get_next_instruction_name(),
                    op=ALU.mult,
                    ins=[ve.lower_ap(e2),
                         ve.lower_ap(eb[hf][0:TC, h, :, :])],
                    outs=[ve.lower_ap(e2)],
                ))

        def denom_recip(h, ee):
            # Denominator for one head, self-broadcasting (see module doc).
            dps = ps_mm.tile([128, NH2], f32, name="dps", tag="psmm")
            for c in range(4):
                nc.tensor.matmul(
                    dps[:, :], lhsT=ones[0:TC, :], rhs=ee[0:TC, c, :],
                    start=(c == 0), stop=(c == 3))
            rbc = rbc_pool.tile([128, NH2], f32, name="rbc", tag="rbc")
            nc.vector.reciprocal_approx_fast(out=rbc[:], in_=dps[:])
            return rbc

        def av_head(h, ee, rbc, avn):
            # out^T = V^T @ ee for one head, then scale by 1/denom while
            # evacuating PSUM into the head's slice of the big avn tile.
            ps = ps_av.tile([128, 2, NH2], f32, name="psav", tag="psav")
            for dd in range(2):
                for tb in range(4):
                    nc.tensor.matmul(
                        ps[:, dd, :],
                        lhsT=vv[0:TC, h, tb, dd * 128:(dd + 1) * 128],
                        rhs=ee[0:TC, tb, :],
                        start=(tb == 0), stop=(tb == 3))
            for dd in range(2):
                nc.vector.scalar_tensor_tensor(
                    avn[:, 2 * h + dd, :], ps[:, dd, :], 0.0, rbc[:],
                    op0=ALU.bypass, op1=ALU.mult)

        def proj_m(pend, m):
            outg, fin = pend[0], pend[1]
            ps = ps_mm.tile([128, NH2], f32, name="psp", tag="psmm")
            for kc in range(16):
                nc.tensor.matmul(
                    ps[:],
                    lhsT=wp[:, kc, m * 128:(m + 1) * 128],
                    rhs=outg[:, kc, :],
                    start=(kc == 0), stop=(kc == 15))
            nc.vector.tensor_scalar(fin[:, m, :], ps[:], bp[:, m:m + 1], None,
                                    op0=ALU.add)

        def qproj_m(xt, m, qt_list):
            psq = ps_mm.tile([128, NH2], f32, name="psq", tag="psmm")
            for kc in range(4):
                nc.tensor.matmul(
                    psq[:],
                    lhsT=wq[:, kc, m * 128:(m + 1) * 128],
                    rhs=xt[:, kc, :],
                    start=(kc == 0), stop=(kc == 3))
            q = qt_pool.tile([128, NH2], bf16, name=f"qt{m}", tag="qt")
            nc.vector.tensor_scalar(q[:], psq[:], bq[:, m:m + 1], None,
                                    op0=ALU.add)
            qt_list.append(q)

        iters = [(b, hf) for b in range(B_LOC) for hf in range(2)]

        # Prologue: Q projection for iters 0 and 1 (the per-iter loop
        # computes qt two iterations ahead, so iteration 0 -- which has no
        # proj work to fill its softmax phase -- still has dense PE slots).
        qt = []
        for m in range(4):
            qproj_m(xts[0], m, qt)
        qt1 = []
        for m in range(4):
            qproj_m(xts[1], m, qt1)

        pends = []      # completed iters' (outg, fin) awaiting projection
        for it, (b, hf) in enumerate(iters):
            qt_next2 = []
            ees = {}        # head -> ee tile
            avn = avn_pool.tile([128, 16, NH2], bf16, name="avn", tag="avn")
            fin = fin_pool.tile([128, 4, NH2], bf16, name="fin", tag="fin")
            nxt = it + 2 < len(iters)
            # iter 1 skips projection (wp is still in flight on the DMA
            # ring); iter 2 drains both pending sets.
            todo = [] if it == 1 else list(pends)
            for hp in range(4):
                pst = ps_sc.tile([128, 2, 2, NH2], f32, name="pst", tag="pssc")
                eep = []
                for i in range(2):
                    t = ee_pool.tile([TC, 4, NH2], bf16, name=f"ee{i}", tag="ee")
                    eep.append(t)
                    ees[2 * hp + i] = t
                # Slot 0 front-loads exp/gelu-independent PE work (the
                # iter+2 Q-proj rounds) so the PE has something to chew on
                # while the ACT drains the previous iter's gelu block and
                # reloads the exp table; proj rounds shift one slot later.
                sc_mms(hp, qt, pst, 0)
                if hp == 0:
                    if nxt:
                        qproj_m(xts[it + 2], 0, qt_next2)
                        qproj_m(xts[it + 2], 1, qt_next2)
                else:
                    for pd in todo:
                        proj_m(pd, hp - 1)
                sc_exp(pst, 0, eep)
                sc_mms(hp, qt, pst, 1)
                if hp == 0 and nxt:
                    qproj_m(xts[it + 2], 2, qt_next2)
                    qproj_m(xts[it + 2], 3, qt_next2)
                sc_exp(pst, 1, eep)
                for i in range(2):
                    ee_mult(2 * hp + i, hf, eep[i])
                if hp >= 1:
                    for i in range(2):
                        h = 2 * (hp - 1) + i
                        av_head(h, ees[h], denom_recip(h, ees[h]), avn)
            for pd in todo:
                proj_m(pd, 3)
                nc.sync.dma_start(out=out_d[pd[2], pd[3]], in_=pd[1][:])

            # GELU phase (own ACT table set), split around the last heads'
            # denominator/AV work so the PE bridges the ACT gelu block; proj
            # consumes outg slices [128, kc, :] directly.
            outg = outg_pool.tile([128, 16, NH2], bf16, name="og", tag="outg")
            for q4 in range(3):
                act(outg[:, 4 * q4:4 * q4 + 4, :], avn[:, 4 * q4:4 * q4 + 4, :],
                    AF.Gelu)
            av_head(6, ees[6], denom_recip(6, ees[6]), avn)
            act(outg[:, 12:14, :], avn[:, 12:14, :], AF.Gelu)
            av_head(7, ees[7], denom_recip(7, ees[7]), avn)
            act(outg[:, 14:16, :], avn[:, 14:16, :], AF.Gelu)
            pends = [p for p in pends if p not in todo] + [(outg, fin, b, hf)]

            qt = qt1
            qt1 = qt_next2
            if it + 5 < len(iters):
                bn, hfn = iters[it + 5]
                t = xt_pool.tile([128, 4, NH2], bf16, name="xt", tag="xt")
                nc.sync.dma_start(out=t[:], in_=xT_d[bn, hfn])
                xts[it + 5] = t

        # Epilogue: the final iteration's proj starts on the gelu slices
        # that are already done (kc 0..11) and finishes after the last gelu.
        outg, pf = pends[-1][0], pends[-1][1]
        pstE = ps_sc.tile([128, 2, 2, NH2], f32, name="pstE", tag="pssc")
        for m in range(4):
            ps = pstE[:, m // 2, m % 2, :]
            for kc in range(12):
                nc.tensor.matmul(ps, lhsT=wp[:, kc, m * 128:(m + 1) * 128],
                                 rhs=outg[:, kc, :],
                                 start=(kc == 0), stop=False)
        for m in range(4):
            ps = pstE[:, m // 2, m % 2, :]
            for kc in range(12, 16):
                nc.tensor.matmul(ps, lhsT=wp[:, kc, m * 128:(m + 1) * 128],
                                 rhs=outg[:, kc, :],
                                 start=False, stop=(kc == 15))
            nc.vector.tensor_scalar(pf[:, m, :], ps, bp[:, m:m + 1], None,
                                    op0=ALU.add)
        pb, phf = iters[-1]
        nc.sync.dma_start(out=out_d[pb, phf], in_=pf[:])

    nc.compile()
    return nc


def _prep_inputs(x, text, q_w, q_gamma, q_beta, q_mean, q_var,
                 kv_w, kv_gamma, kv_beta, kv_mean, kv_var,
                 proj_w, proj_gamma, proj_beta, proj_mean, proj_var,
                 attention_biases):
    """Host-side constant folding + layout prep. Returns per-core in_maps."""
    scale = KEY_DIM ** -0.5

    # Fold q BN + softmax scale into the q weight/bias.
    s_q = q_gamma / np.sqrt(q_var + EPS)
    wq_eff = (q_w * s_q[None, :] * scale).astype(np.float32)
    wq_pack = np.ascontiguousarray(
        wq_eff.reshape(4, 128, NH_KD).transpose(1, 0, 2)
    ).astype(ml_dtypes.bfloat16)
    bq_eff = ((q_beta - q_mean * s_q) * scale).astype(np.float32)
    bq_pack = np.ascontiguousarray(bq_eff.reshape(4, 128).T).astype(np.float32)

    # kv projection on host (shared across batch; ~1/150 of total FLOPs).
    s_kv = kv_gamma / np.sqrt(kv_var + EPS)
    kv = (text @ kv_w - kv_mean[None, :]) * s_kv[None, :] + kv_beta[None, :]
    kv = kv.astype(np.float32).reshape(NT, NUM_HEADS, KEY_DIM + D_V)
    k = kv[:, :, :KEY_DIM]          # (NT, H, KD)
    v = kv[:, :, KEY_DIM:]          # (NT, H, DV)
    # kT: [128 (pair-stacked kd), hp, NT]
    kT = k.transpose(1, 2, 0).reshape(4, 128, NT).transpose(1, 0, 2)
    kT = np.ascontiguousarray(kT).astype(ml_dtypes.bfloat16)
    # v: [t_local(100), h, chunk(4), dv]
    v_pack = np.ascontiguousarray(
        v.reshape(4, TC, NUM_HEADS, D_V).transpose(1, 2, 0, 3)
    ).astype(ml_dtypes.bfloat16)

    # exp() of the gathered relative position bias ->
    # [half, t_local(100), h, chunk(4), n(512)] bf16.
    n = np.arange(H_GRID * W_GRID)
    i, j = n // W_GRID, n % W_GRID
    t = np.arange(NT)
    a, bb = t // 100, t % 100
    idxs = np.abs(i[:, None] - a[None, :]) * 100 + np.abs(j[:, None] - bb[None, :])
    bias = attention_biases[:, idxs]                  # (H, N, NT) f32
    # (H, N, NT) -> [hf, t_local, h, chunk, n]
    biasT = bias.reshape(NUM_HEADS, 2, NH2, 4, TC).transpose(1, 4, 0, 3, 2)
    expbias = np.exp(np.ascontiguousarray(biasT)).astype(ml_dtypes.bfloat16)

    # Fold proj BN scale into wp, shift stays as epilogue bias.
    s_p = proj_gamma / np.sqrt(proj_var + EPS)
    wp_eff = (proj_w * s_p[None, :]).astype(np.float32)
    wp_pack = np.ascontiguousarray(
        wp_eff.reshape(16, 128, DIM).transpose(1, 0, 2)
    ).astype(ml_dtypes.bfloat16)
    bp_eff = (proj_beta - proj_mean * s_p).astype(np.float32)
    bp_pack = np.ascontiguousarray(bp_eff.reshape(4, 128).T).astype(np.float32)

    shared = {
        "wq": wq_pack, "bq": bq_pack, "kT": kT, "v": v_pack,
        "expbias": expbias, "wp": wp_pack, "bp": bp_pack,
    }
    in_maps = []
    for c in range(N_CORES):
        xs = x[c * B_LOC:(c + 1) * B_LOC]                       # (4, N, DIM)
        # [b, hf, p(128), kc(4), n(512)]
        xT = xs.transpose(0, 2, 1).reshape(B_LOC, 4, 128, 2, NH2)
        xT = np.ascontiguousarray(xT.transpose(0, 3, 2, 1, 4))
        m = dict(shared)
        m["xT"] = xT.astype(ml_dtypes.bfloat16)
        in_maps.append(m)
    return in_maps


def kernel(x, text, q_w, q_gamma, q_beta, q_mean, q_var,
           kv_w, kv_gamma, kv_beta, kv_mean, kv_var,
           proj_w, proj_gamma, proj_beta, proj_mean, proj_var,
           attention_biases, H, W, **_unused):
    from concourse.bass_utils import run_bass_kernel_spmd

    x = np.asarray(x, dtype=np.float32)
    in_maps = _prep_inputs(
        np.asarray(x, np.float32), np.asarray(text, np.float32),
        np.asarray(q_w, np.float32), np.asarray(q_gamma, np.float32),
        np.asarray(q_beta, np.float32), np.asarray(q_mean, np.float32),
        np.asarray(q_var, np.float32),
        np.asarray(kv_w, np.float32), np.asarray(kv_gamma, np.float32),
        np.asarray(kv_beta, np.float32), np.asarray(kv_mean, np.float32),
        np.asarray(kv_var, np.float32),
        np.asarray(proj_w, np.float32), np.asarray(proj_gamma, np.float32),
        np.asarray(proj_beta, np.float32), np.asarray(proj_mean, np.float32),
        np.asarray(proj_var, np.float32),
        np.asarray(attention_biases, np.float32))

    if "nc" not in _CACHE:
        _CACHE["nc"] = _build_nc()
    nc = _CACHE["nc"]

    res = run_bass_kernel_spmd(nc, in_maps, list(range(N_CORES)))
    outs = [np.asarray(res.results[c]["outT"], dtype=np.float32)
            for c in range(N_CORES)]               # (4, 2, 128, 4, 512)
    full = np.concatenate(outs, axis=0)            # (B, 2, 128, 4, 512)
    # out[b, hf, p, m, n] = final[dim=m*128+p, tok=hf*512+n]
    full = full.transpose(0, 1, 4, 3, 2).reshape(B, N_TOK, DIM)
    return np.ascontiguousarray(full)
